# revision 21
# baseline (speedup 1.0000x reference)
"""Trainium2 Bass kernel for nn_EnhancedDepthwiseSeparableFFN (v2).

Data-parallel over the batch: 8 samples -> 8 NeuronCores, one sample each.
Cross-core traffic: three tiny AllGathers for the BatchNorm batch statistics.

v2 changes over the baseline (243us):
  - bf16 everywhere on the PE path (FWL weight loads, full-rate stencil
    pieces, cheap LDWEIGHTS) and for the big elementwise tensors (DVE 2x/4x).
  - double-expand: the spatial-major gelu1 output is recomputed as a second
    expand matmul pass (overlapping the AG1 collective) instead of 64 PE
    transposes + PSUM copies.
  - stats ride fused ops: ACT accum (sums) + DVE tensor_tensor_reduce
    (sum-of-squares); per-channel max moved to the idle GPSIMD engine.
  - BN1/BN2 rsqrt via the bit-trick + 2 Newton steps on DVE (no ACT Sqrt
    table loads between the Gelu phases); ACT tables are preloaded with
    dummy ops so Exp/Gelu/Sigmoid loads hide under collective waits.
  - srow from a host-side tap-indicator matrix (one small matmul);
    b1row via 8 tiny column transposes (no DRAM bounce).
  - final tail split across DVE and GPSIMD.
"""
import numpy as np

import concourse.bass as bass
import concourse.bacc as bacc
import concourse.tile as tile
from concourse import mybir, bass_utils, bass_isa

F32 = mybir.dt.float32
BF16 = mybir.dt.bfloat16
U32 = mybir.dt.uint32
AF = mybir.ActivationFunctionType
OP = mybir.AluOpType

NP_BF16 = mybir.dt.np(BF16)

D = 256          # model dim
C = 1024         # expanded channels
H = W = 32
HW = 1024
NCORES = 8
B = 8            # batch
EPS = 1e-5
CT = C // 128    # 8 channel tiles
HT = HW // 128   # 8 spatial tiles
RSQRT_ITERS = 2


# ---------------------------------------------------------------- host consts

def _stencil_masks():
    """(128, 9*384) f32: per q a full [L(-1) | L(0) | L(+1)] contribution
    mask, so L = sum_q kw[q] * M_q builds in 9 wide DVE ops.

    L_delta[k_in, m_out] = kw[q],  q = (dh+1)*3 + (dw+1),
    dh = h_in - h_out = r_in - r_out - 4*delta, dw = w_in - w_out.
    """
    k = np.arange(128)
    m = np.arange(128)
    r_in, w_in = k // 32, k % 32
    r_out, w_out = m // 32, m % 32
    dw = w_in[:, None] - w_out[None, :]
    out = np.zeros((9, 128, 384), np.float32)
    for bi, delta in enumerate((-1, 0, 1)):
        dh = r_in[:, None] - r_out[None, :] - 4 * delta
        for q in range(9):
            dh_q, dw_q = q // 3 - 1, q % 3 - 1
            out[q, :, bi * 128:(bi + 1) * 128] = (
                (dh == dh_q) & (dw == dw_q)).astype(np.float32)
    # delta=-1 only contributes taps q in {6,7,8}; delta=+1 only {0,1,2}
    for q in range(9):
        if q < 6:
            out[q, :, 0:128] = 0.0
        if q > 2:
            out[q, :, 256:384] = 0.0
    return np.concatenate(list(out), axis=1)  # (128, 9*384)


def _spatial_bands(sw):
    """(128, 6*128) f32 lhsT band tiles for the 7x7 conv, cols =
    [ch0 d-1,d0,d+1 | ch1 d-1,d0,d+1]."""
    k = np.arange(128)
    m = np.arange(128)
    r_in, w_in = k // 32, k % 32
    r_out, w_out = m // 32, m % 32
    dw = w_in[:, None] - w_out[None, :]
    wok = np.abs(dw) <= 3
    tiles = []
    for ch in range(2):
        for delta in (-1, 0, 1):
            dh = r_in[:, None] - r_out[None, :] - 4 * delta
            hok = np.abs(dh) <= 3
            t = np.zeros((128, 128), np.float32)
            ok = hok & wok
            t[ok] = sw[0, ch][(dh[ok] + 3, dw[ok] + 3)]
            tiles.append(t)
    return np.concatenate(tiles, axis=1)  # (128, 6*128)


def _tap_counts():
    """(9, 1024) f32: SB9[q, px] = 1 if 3x3 tap q is in-bounds at pixel px.

    srow = kw @ SB9 gives the per-pixel sum of present tap weights.
    """
    px = np.arange(HW)
    h, w = px // W, px % W
    out = np.zeros((9, HW), np.float32)
    for q in range(9):
        dh, dw = q // 3 - 1, q % 3 - 1
        ok = (h + dh >= 0) & (h + dh < H) & (w + dw >= 0) & (w + dw < W)
        out[q] = ok.astype(np.float32)
    return out


# ---------------------------------------------------------------- the program

def build_program(sw, sim_gelu_identity=False, n_cores=NCORES, debug=False):
    gelu_f = AF.Identity if sim_gelu_identity else AF.Gelu

    nc = bacc.Bacc("TRN2", target_bir_lowering=False, debug=False,
                   num_devices=n_cores)

    I = {}
    for name, shape, dt in [
        ("x", [HW, D], F32),
        ("xt", [D, HW], BF16), ("w1t", [D, C], BF16),
        ("b1r", [1, C], BF16),
        ("b1c", [128, CT], F32),
        ("g1c", [128, CT], F32), ("be1c", [128, CT], F32),
        ("g2c", [128, CT], F32), ("be2c", [128, CT], F32),
        ("aw1t", [C, 128], BF16), ("ab1c", [128, 1], F32),
        ("aw2t", [128, 9], BF16), ("ab2r", [1, 9], F32),
        ("caw1t", [C, 64], BF16), ("caw2t", [64, C], BF16),
        ("pwt", [C, D], BF16),
        ("g3r", [1, D], F32), ("be3r", [1, D], F32),
        ("sbr", [1, 1], F32),
    ]:
        I[name] = nc.dram_tensor(name, shape, dt, kind="ExternalInput")
    out_d = nc.dram_tensor("out", [HW, D], F32, kind="ExternalOutput")

    ident = nc.inline_tensor(np.eye(128, dtype=np.float32), name="cident")
    ident1 = nc.inline_tensor(np.ones((1, 1), np.float32), name="cident1")
    ones_row = nc.inline_tensor(np.ones((1, 128), np.float32), name="conesr")
    ones_row_bf = nc.inline_tensor(np.ones((1, 128), NP_BF16), name="conesrb")
    ones_col = nc.inline_tensor(np.ones((128, 1), np.float32), name="conesc")
    ones_col_bf = nc.inline_tensor(np.ones((128, 1), NP_BF16), name="conescb")
    masks_c = nc.inline_tensor(_stencil_masks().astype(NP_BF16), name="cmasks")
    spb_c = nc.inline_tensor(_spatial_bands(sw), name="cspb")
    sb9_c = nc.inline_tensor(_tap_counts().astype(NP_BF16), name="csb9")
    magic_c = nc.inline_tensor(
        np.full((128, 8), 0x5f3759df, np.uint32), name="cmagic")
    one_u32_c = nc.inline_tensor(np.full((128, 8), 1, np.uint32), name="cone32")

    with tile.TileContext(nc) as tc:
        _body(nc, tc, I, out_d, ident, ident1, ones_row, ones_row_bf,
              ones_col, ones_col_bf, masks_c, spb_c, sb9_c, magic_c,
              one_u32_c, gelu_f, n_cores)
    nc.compile()
    return nc


def _rsqrt_cols(nc, sb, magic, one32, v, n, tag):
    """DVE-only rsqrt of v (128, n) f32 (in place OK) -> returns (128, n).

    Bit-trick initial guess + 2 Newton steps; rel err ~5e-6.
    """
    y = sb.tile([128, n], F32, tag=tag + "y")
    t = sb.tile([128, n], F32, tag=tag + "t")
    yu = y[:].bitcast(U32)
    nc.vector.tensor_tensor(yu, v[:].bitcast(U32), one32[:, 0:n],
                            OP.logical_shift_right)
    nc.vector.tensor_tensor(yu, magic[:, 0:n], yu, OP.subtract)
    for _ in range(RSQRT_ITERS):
        nc.vector.tensor_tensor(t[:], y[:], y[:], OP.mult)
        nc.vector.tensor_tensor(t[:], t[:], v[:], OP.mult)
        nc.vector.tensor_scalar(t[:], t[:], -0.5, 1.5, OP.mult, OP.add)
        nc.vector.tensor_tensor(y[:], y[:], t[:], OP.mult)
    return y


def _body(nc, tc, I, out_d, ident, ident1, ones_row, ones_row_bf, ones_col,
          ones_col_bf, masks_c, spb_c, sb9_c, magic_c, one_u32_c, gelu_f,
          n_cores=NCORES):
    nb = n_cores * HW

    with tc.tile_pool(name="sb", bufs=1) as sb, \
         tc.tile_pool(name="sb2", bufs=1) as sb2, \
         tc.tile_pool(name="psb", bufs=3, space="PSUM") as psB, \
         tc.tile_pool(name="psh", bufs=2, space="PSUM") as psH, \
         tc.tile_pool(name="dram", bufs=6, space="DRAM") as dram:

        def load(name, shape, ap_in, dt=F32, pool=sb, view=None):
            t = pool.tile(shape, dt, tag=name)
            dst = t[:] if view is None else t[:].rearrange(*view[0], **view[1])
            nc.sync.dma_start(dst, ap_in)
            return t

        # ---------------- persistent SBUF tensors + loads
        # (ordered by first use: phase-1 operands first so the expand
        # matmuls are not stuck behind late-needed bulk transfers)
        tid1 = load("tid1", [1, 1], ident1.ap())
        b1c = load("b1c", [128, CT], I["b1c"].ap())
        xt_sb = sb.tile([128, 2 * HW], BF16, tag="xt_sb")
        w1t_sb = sb.tile([128, 2 * C], BF16, tag="w1t_sb")
        for k in range(2):
            nc.sync.dma_start(
                xt_sb[:, k * HW:(k + 1) * HW],
                I["xt"].ap().rearrange("(k p) n -> p k n", p=128)[:, k, :])
            nc.sync.dma_start(
                w1t_sb[:, k * C:(k + 1) * C],
                I["w1t"].ap().rearrange("(k p) n -> p k n", p=128)[:, k, :])
        tonesrb = load("tonesrb", [1, 128], ones_row_bf.ap(), dt=BF16)
        b1rb = load("b1rb", [1, C], I["b1r"].ap(), dt=BF16)
        magic = load("magic", [128, 8], magic_c.ap(), dt=U32)
        one32 = load("one32", [128, 8], one_u32_c.ap(), dt=U32)
        g1c = load("g1c", [128, CT], I["g1c"].ap())
        be1c = load("be1c", [128, CT], I["be1c"].ap())
        ab1c = load("ab1c", [128, 1], I["ab1c"].ap())
        aw1t_sb = load("aw1t_sb", [128, 8 * 128],
                       I["aw1t"].ap().rearrange("(k p) n -> p k n", p=128),
                       dt=BF16, view=(("p (k n) -> p k n",), dict(k=8)))
        aw2t_sb = load("aw2t_sb", [128, 9], I["aw2t"].ap(), dt=BF16)
        ab2r = load("ab2r", [1, 9], I["ab2r"].ap())
        tid = load("tid", [128, 128], ident.ap())
        tonesr = load("tonesr", [1, 128], ones_row.ap())
        masks = load("masks", [128, 9 * 384], masks_c.ap(), dt=BF16)
        sb9 = load("sb9", [9, HW], sb9_c.ap(), dt=BF16)
        g2c = load("g2c", [128, CT], I["g2c"].ap())
        be2c = load("be2c", [128, CT], I["be2c"].ap())
        caw1t_sb = load("caw1t_sb", [128, 8 * 64],
                        I["caw1t"].ap().rearrange("(k p) n -> p k n", p=128),
                        dt=BF16, view=(("p (k n) -> p k n",), dict(k=8)))
        caw2t_sb = load("caw2t_sb", [64, C], I["caw2t"].ap(), dt=BF16)
        paug = sb.tile([128, 8 * 260], BF16, tag="paug")
        nc.sync.dma_start(
            paug[:].rearrange("p (k n) -> p k n", n=260)[:, :, 0:D],
            I["pwt"].ap().rearrange("(k p) n -> p k n", p=128))
        nc.vector.memset(
            paug[:].rearrange("p (k n) -> p k n", n=260)[:, :, D:D + 1],
            1.0 / C)
        nc.vector.memset(
            paug[:].rearrange("p (k n) -> p k n", n=260)[:, :, D + 1:D + 2],
            0.0)
        tonesc = load("tonesc", [128, 1], ones_col.ap())
        tonescb = load("tonescb", [128, 1], ones_col_bf.ap(), dt=BF16)
        spb = load("spb", [128, 6 * 128], spb_c.ap())
        g3r = load("g3r", [1, D], I["g3r"].ap())
        be3r = load("be3r", [1, D], I["be3r"].ap())
        sbr = load("sbr", [1, 1], I["sbr"].ap())
        xres = load("xres", [128, 8 * D],
                    I["x"].ap().rearrange("(t p) d -> p t d", p=128),
                    view=(("p (t d) -> p t d",), dict(t=8)))

        # big working tensors (bf16)
        yg_sp = sb2.tile([128, HT * C], BF16, tag="ygsp")   # gelu1, spatial-major
        g2o = sb2.tile([128, CT * HW], BF16, tag="g2o")     # gelu2, ch-major
        yca = sb2.tile([128, CT * HW], BF16, tag="yca")     # ch-att out
        ygscr = [sb.tile([128, HW], BF16, tag=f"ygscr{i}", name=f"ygscr{i}")
                 for i in range(2)]
        sqscr = [sb.tile([128, HW], BF16, tag=f"sqscr{i}", name=f"sqscr{i}")
                 for i in range(2)]
        stat1l = sb.tile([128, 16], F32, tag="stat1l")
        stat1g = sb.tile([128, 16], F32, tag="stat1g")
        stat2l = sb.tile([128, 16], F32, tag="stat2l")
        stat2g = sb.tile([128, 16], F32, tag="stat2g")
        dscr = sb.tile([1, 8], F32, tag="dscr")             # ACT table preload dst

        # table preload: gelu load hides under the input DMAs
        nc.scalar.activation(dscr[:, 0:1], tid1[:], gelu_f, bias=0.0,
                             scale=1.0)

        # ============================ PHASE 1a: expand (ch-major) + stats1
        for m in range(CT):
            ps = psB.tile([128, HW], F32, tag="psb")
            for k in range(2):      # k outer: one LDWEIGHTS per (m, k)
                for h in range(2):
                    nc.tensor.matmul(
                        ps[:, h * 512:(h + 1) * 512],
                        w1t_sb[:, k * C + m * 128: k * C + (m + 1) * 128],
                        xt_sb[:, k * HW + h * 512: k * HW + (h + 1) * 512],
                        start=(k == 0), stop=(k == 1))
            yscr = ygscr[m % 2]
            nc.scalar.activation(
                yscr[:], ps[:], gelu_f, bias=b1c[:, m:m + 1], scale=1.0,
                accum_out=stat1l[:, m:m + 1])
            nc.vector.scalar_tensor_tensor(
                sqscr[m % 2][:], yscr[:], 0.0, yscr[:], OP.bypass, OP.mult,
                accum_out=stat1l[:, 8 + m:9 + m])

        # ============================ AG1 (BN1 batch stats)
        bb1i = dram.tile([128, 16], F32, tag="bb1i")
        bb1o = dram.tile([n_cores * 128, 16], F32, tag="bb1o")
        nc.gpsimd.dma_start(bb1i[:], stat1l[:])
        nc.gpsimd.collective_compute(
            "AllGather", OP.bypass, replica_groups=[list(range(n_cores))],
            ins=[bb1i.opt()], outs=[bb1o.opt()])

        # ============================ PHASE 1b: expand again, spatial-major
        # (runs on PE/ACT while the AG1 collective is in flight)
        for t in range(HT):
            ps2 = psB.tile([128, HW], F32, tag="psb")
            for k in range(2):      # k outer: one LDWEIGHTS per (t, k)
                for g in range(2):
                    nc.tensor.matmul(
                        ps2[:, g * 512:(g + 1) * 512],
                        xt_sb[:, k * HW + t * 128: k * HW + (t + 1) * 128],
                        w1t_sb[:, k * C + g * 512: k * C + (g + 1) * 512],
                        start=(k == 0), stop=False)
            for g in range(2):
                nc.tensor.matmul(
                    ps2[:, g * 512:(g + 1) * 512],
                    tonesrb[:],
                    b1rb[:, g * 512:(g + 1) * 512],
                    start=False, stop=True)
            nc.scalar.activation(
                yg_sp[:, t * C:(t + 1) * C], ps2[:], gelu_f,
                bias=0.0, scale=1.0)
        # preload the Exp table right after the last sp-pass gelu (the
        # data dep pins it there; the load hides under the AG1 flight)
        nc.scalar.activation(dscr[:, 1:2], yg_sp[0:1, HT * C - 1:HT * C],
                             AF.Exp, bias=0.0, scale=1.0)

        # gather AG1 result + local combine
        gath1 = sb.tile([128, n_cores * 16], F32, tag="gath1")
        nc.gpsimd.dma_start(
            gath1[:].rearrange("p (r f) -> p r f", f=16),
            bb1o[:].rearrange("(r p) f -> p r f", p=128))
        nc.vector.tensor_reduce(
            stat1g[:], gath1[:].rearrange("p (r f) -> p f r", f=16),
            mybir.AxisListType.X, OP.add)

        # ============================ PHASE 3: BN1 affine + kw + L build
        def bn_affine(statg, gcol, becol, tagp):
            """-> (a, bn) per-channel scale/shift columns (128, CT)."""
            mns = sb.tile([128, CT], F32, tag=tagp + "m")
            var = sb.tile([128, CT], F32, tag=tagp + "v")
            a = sb.tile([128, CT], F32, tag=tagp + "a")
            bn = sb.tile([128, CT], F32, tag=tagp + "b")
            nc.vector.tensor_scalar_mul(mns[:], statg[:, 0:8], 1.0 / nb)
            nc.vector.tensor_tensor(var[:], mns[:], mns[:], OP.mult)
            nc.vector.scalar_tensor_tensor(
                var[:], statg[:, 8:16], 1.0 / nb, var[:], OP.mult, OP.subtract)
            nc.vector.tensor_scalar_add(var[:], var[:], EPS)
            rs = _rsqrt_cols(nc, sb, magic, one32, var, CT, tagp + "r")
            nc.vector.tensor_tensor(a[:], gcol[:], rs[:], OP.mult)
            nc.vector.tensor_tensor(bn[:], mns[:], a[:], OP.mult)
            nc.vector.tensor_tensor(bn[:], becol[:], bn[:], OP.subtract)
            return a, bn

        a1, b1n = bn_affine(stat1g, g1c, be1c, "s1")
        inva1 = sb.tile([128, CT], F32, tag="inva1")
        bpre = sb.tile([128, CT], F32, tag="bpre")
        nc.vector.reciprocal(inva1[:], a1[:])
        nc.vector.tensor_tensor(bpre[:], b1n[:], inva1[:], OP.mult)
        # b' row (1, C) via 8 tiny column transposes (no DRAM bounce)
        b1rowb = sb.tile([1, C], BF16, tag="b1rowb")
        for half in range(2):
            psb1 = psH.tile([1, 512], F32, tag="psh")
            for j in range(4):
                c = half * 4 + j
                nc.tensor.transpose(psb1[:, j * 128:(j + 1) * 128],
                                    bpre[:, c:c + 1], tid[:])
            nc.vector.tensor_copy(b1rowb[:, half * 512:(half + 1) * 512], psb1[:])

        # gap (local, normalized) -> kw
        gapn = sb.tile([128, CT], F32, tag="gapn")
        gapb = sb.tile([128, CT], BF16, tag="gapb")
        nc.vector.scalar_tensor_tensor(
            gapn[:], stat1l[:, 0:8], 1.0 / HW, a1[:], OP.mult, OP.mult)
        nc.vector.tensor_tensor(gapn[:], gapn[:], b1n[:], OP.add)
        nc.vector.tensor_copy(gapb[:], gapn[:])

        # PE keep-warm: dummy matmuls with post-AG1 deps so HAM does not
        # re-throttle during the mostly-serial kw chain
        for i in range(4):
            psw = psH.tile([1, 512], F32, tag="psh")
            nc.tensor.matmul(psw[:], gapb[:, 0:1],
                             xt_sb[:, i * 512:(i + 1) * 512],
                             start=True, stop=True)
        ph1 = psH.tile([128, 1], F32, tag="psh")
        for k in range(CT):
            nc.tensor.matmul(ph1[:], aw1t_sb[:, k * 128:(k + 1) * 128],
                             gapb[:, k:k + 1], start=(k == 0), stop=(k == 7))
        h1 = sb.tile([128, 1], BF16, tag="h1")
        nc.vector.tensor_scalar(h1[:], ph1[:], ab1c[:], 0.0, OP.add, OP.max)
        ps9 = psH.tile([1, 9], F32, tag="psh")
        nc.tensor.matmul(ps9[:], h1[:], aw2t_sb[:], start=True, stop=True)
        v9 = sb.tile([1, 9], F32, tag="v9")
        nc.vector.tensor_tensor(v9[:], ps9[:], ab2r[:], OP.add)
        mx9 = sb.tile([1, 1], F32, tag="mx9")
        nc.vector.tensor_reduce(mx9[:], v9[:], mybir.AxisListType.X, OP.max)
        nc.vector.tensor_scalar(v9[:], v9[:], mx9[:], None, OP.subtract)
        e9 = sb.tile([1, 9], F32, tag="e9")
        se = sb.tile([1, 1], F32, tag="se")
        nc.scalar.activation(e9[:], v9[:], AF.Exp, bias=0.0, scale=1.0,
                             accum_out=se[:])
        # re-preload Gelu for phase 4 (dep on the exp output pins it after
        # the real Exp; the load hides under the kw/L-build chain)
        nc.scalar.activation(dscr[:, 2:3], e9[:, 0:1], gelu_f, bias=0.0,
                             scale=1.0)
        rse = sb.tile([1, 1], F32, tag="rse")
        nc.vector.reciprocal(rse[:], se[:])
        kw9 = sb.tile([1, 9], F32, tag="kw9")
        nc.vector.tensor_scalar(kw9[:], e9[:], rse[:], None, OP.mult)
        # broadcast kw to all partitions (for the L build scalars)
        pskb = psH.tile([128, 9], F32, tag="psh")
        nc.tensor.matmul(pskb[:], tonesr[:], kw9[:], start=True, stop=True)
        kwb = sb.tile([128, 9], F32, tag="kwb")
        nc.vector.tensor_copy(kwb[:], pskb[:])
        kwbb = sb.tile([128, 1], BF16, tag="kwbb")
        nc.vector.tensor_copy(kwbb[:], kwb[:, 0:1])
        for i in range(4):
            psw = psH.tile([1, 512], F32, tag="psh")
            nc.tensor.matmul(psw[:], kwbb[:],
                             xt_sb[:, i * 512:(i + 1) * 512],
                             start=True, stop=True)
        # kw as a column (9, 1) for the srow matmul
        pskc = psH.tile([9, 1], F32, tag="psh")
        nc.tensor.transpose(pskc[:], kw9[:], tid1[:])
        kwcol = sb.tile([9, 1], BF16, tag="kwcol")
        nc.vector.tensor_copy(kwcol[:], pskc[:])

        # L band tiles, concatenated [L(-1) | L(0) | L(+1)]: 9 wide DVE ops
        L = sb.tile([128, 3 * 128], BF16, tag="L")
        nc.vector.tensor_scalar(L[:], masks[:, 0:384], kwb[:, 0:1],
                                None, OP.mult)
        for q in range(1, 9):
            nc.vector.scalar_tensor_tensor(
                L[:], masks[:, q * 384:(q + 1) * 384], kwb[:, q:q + 1],
                L[:], OP.mult, OP.add)

        # srow = kw @ SB9 (per-pixel sum of present taps)
        srowb = sb.tile([1, HW], BF16, tag="srowb")
        for h in range(2):
            pss = psH.tile([1, 512], F32, tag="psh")
            nc.tensor.matmul(pss[:], kwcol[:],
                             sb9[:, h * 512:(h + 1) * 512],
                             start=True, stop=True)
            nc.vector.tensor_copy(srowb[:, h * 512:(h + 1) * 512], pss[:])

        # ============================ PHASE 4: stencil + gelu2 + stats2
        # rank-1 opens are emitted up to 3 tiles ahead so the PE can run
        # them (and warm up) while the DVE is still building L
        mxc = sb.tile([128, CT], F32, tag="mxc")
        psz_q = {}

        def open_psz(c):
            psz = psB.tile([128, HW], F32, tag="psb")
            for h in range(2):
                nc.tensor.matmul(psz[:, h * 512:(h + 1) * 512],
                                 b1rowb[:, c * 128:(c + 1) * 128],
                                 srowb[:, h * 512:(h + 1) * 512],
                                 start=True, stop=False)
            psz_q[c] = psz

        for c in range(3):
            open_psz(c)
        for c in range(CT):
            psz = psz_q.pop(c)
            for t_in in range(HT):
                lo = max(0, (t_in - 1) * 128)
                hi = min(HW, (t_in + 2) * 128)
                roff = 128 + (lo - t_in * 128)
                if lo < 512 < hi:
                    pieces = [(lo, 512), (512, hi)]
                else:
                    pieces = [(lo, hi)]
                for (a, b) in pieces:
                    ra = roff + (a - lo)
                    last_bank0 = (a < 512) and (t_in == 4)
                    last_bank1 = (a >= 512) and (t_in == 7)
                    nc.tensor.matmul(
                        psz[:, a:b],
                        yg_sp[:, t_in * C + c * 128: t_in * C + (c + 1) * 128],
                        L[:, ra:ra + (b - a)],
                        start=False, stop=(last_bank0 or last_bank1))
            nc.scalar.activation(
                g2o[:, c * HW:(c + 1) * HW], psz[:], gelu_f,
                bias=0.0, scale=a1[:, c:c + 1],
                accum_out=stat2l[:, c:c + 1])
            if c + 3 < CT:
                open_psz(c + 3)
            srcg2 = g2o[:, c * HW:(c + 1) * HW]
            nc.vector.scalar_tensor_tensor(
                sqscr[c % 2][:], srcg2, 0.0, srcg2, OP.bypass, OP.mult,
                accum_out=stat2l[:, 8 + c:9 + c])
            nc.vector.tensor_reduce(mxc[:, c:c + 1], srcg2,
                                    mybir.AxisListType.X, OP.max)

        # ============================ AG2 (BN2 batch stats)
        bb2i = dram.tile([128, 16], F32, tag="bb2i")
        bb2o = dram.tile([n_cores * 128, 16], F32, tag="bb2o")
        nc.gpsimd.dma_start(bb2i[:], stat2l[:])
        nc.gpsimd.collective_compute(
            "AllGather", OP.bypass, replica_groups=[list(range(n_cores))],
            ins=[bb2i.opt()], outs=[bb2o.opt()])
        # preload Sigmoid while AG2 is in flight (dep pins it after the
        # last stencil gelu)
        nc.scalar.activation(dscr[:, 3:4], g2o[0:1, CT * HW - 1:CT * HW],
                             AF.Sigmoid, bias=0.0, scale=1.0)
        gath2 = sb.tile([128, n_cores * 16], F32, tag="gath2")
        nc.gpsimd.dma_start(
            gath2[:].rearrange("p (r f) -> p r f", f=16),
            bb2o[:].rearrange("(r p) f -> p r f", p=128))
        nc.vector.tensor_reduce(
            stat2g[:], gath2[:].rearrange("p (r f) -> p f r", f=16),
            mybir.AxisListType.X, OP.add)

        # ============================ PHASE 6: BN2 + channel attention
        a2, b2n = bn_affine(stat2g, g2c, be2c, "s2")
        amx = sb.tile([128, 2 * CT], F32, tag="amx")
        amxb = sb.tile([128, 2 * CT], BF16, tag="amxb")
        nc.vector.scalar_tensor_tensor(
            amx[:, 0:8], stat2l[:, 0:8], 1.0 / HW, a2[:], OP.mult, OP.mult)
        nc.vector.tensor_tensor(amx[:, 0:8], amx[:, 0:8], b2n[:], OP.add)
        nc.vector.tensor_tensor(amx[:, 8:16], mxc[:], a2[:], OP.mult)
        nc.vector.tensor_tensor(amx[:, 8:16], amx[:, 8:16], b2n[:], OP.add)
        nc.vector.tensor_copy(amxb[:], amx[:])

        for i in range(4):
            psw = psH.tile([1, 512], F32, tag="psh")
            nc.tensor.matmul(psw[:], amxb[:, 0:1],
                             xt_sb[:, i * 512:(i + 1) * 512],
                             start=True, stop=True)
        psf = psH.tile([64, 2], F32, tag="psh")
        for k in range(CT):
            nc.tensor.matmul(psf[:], caw1t_sb[:, k * 64:(k + 1) * 64],
                             amxb[:, k:k + 9:8], start=(k == 0), stop=(k == 7))
        hsum = sb.tile([64, 1], BF16, tag="hsum")
        hp = sb.tile([64, 2], F32, tag="hp")
        nc.vector.tensor_scalar(hp[:], psf[:], 0.0, None, OP.max)
        nc.vector.tensor_tensor(hsum[:], hp[:, 0:1], hp[:, 1:2], OP.add)

        psc = psH.tile([128, CT], F32, tag="psh")
        for c in range(CT):
            nc.tensor.matmul(psc[:, c:c + 1], caw2t_sb[:, c * 128:(c + 1) * 128],
                             hsum[:], start=True, stop=True)
        scol = sb.tile([128, CT], F32, tag="scol")
        nc.scalar.activation(scol[:], psc[:], AF.Sigmoid, bias=0.0, scale=1.0)

        sprime = sb.tile([128, CT], F32, tag="sprime")
        b2s = sb.tile([128, CT], F32, tag="b2s")
        b2sb = sb.tile([128, CT], BF16, tag="b2sb")
        nc.vector.tensor_tensor(sprime[:], scol[:], a2[:], OP.mult)
        nc.vector.tensor_tensor(b2s[:], scol[:], b2n[:], OP.mult)
        nc.vector.tensor_copy(b2sb[:], b2s[:])

        # scaled projection weights first: they gate the projection matmuls
        pws = sb2.tile([128, 8 * 260], BF16, tag="pws")
        for c in range(CT):
            nc.vector.tensor_scalar(pws[:, c * 260:c * 260 + 258],
                                    paug[:, c * 260:c * 260 + 258],
                                    sprime[:, c:c + 1], None, OP.mult)

        # y_ca (materialized for the channel-max)
        for c in range(CT):
            nc.vector.tensor_scalar(yca[:, c * HW:(c + 1) * HW],
                                    g2o[:, c * HW:(c + 1) * HW],
                                    sprime[:, c:c + 1], b2s[:, c:c + 1],
                                    OP.mult, OP.add)
        # t2 row (rank-1 bias of the projection)
        pst2 = psH.tile([1, 258], F32, tag="psh")
        for c in range(CT):
            nc.tensor.matmul(pst2[:], b2sb[:, c:c + 1],
                             paug[:, c * 260:c * 260 + 258],
                             start=(c == 0), stop=(c == 7))
        u2row = sb.tile([1, 258], BF16, tag="u2row")
        nc.vector.tensor_copy(u2row[:], pst2[:])

        # projection -> proj_sb (spatial-major (hw, d))
        proj_sb = sb2.tile([128, 8 * 258], BF16, tag="proj_sb")
        avgpx = sb.tile([128, HT], F32, tag="avgpx")
        for mt in range(HT):
            psp = psH.tile([128, 258], F32, tag="psh")
            for c in range(CT):
                nc.tensor.matmul(psp[:],
                                 g2o[:, c * HW + mt * 128: c * HW + (mt + 1) * 128],
                                 pws[:, c * 260:c * 260 + 258],
                                 start=(c == 0), stop=False)
            nc.tensor.matmul(psp[:], tonesrb[:], u2row[:], start=False,
                             stop=True)
            dst = proj_sb[:, mt * 258:mt * 258 + 258]
            nc.vector.tensor_copy(dst, psp[:])
            nc.vector.tensor_copy(avgpx[:, mt:mt + 1], psp[:, D:D + 1])

        # channel max (per pixel): in-place pairwise tree over yca, then a
        # partition all-reduce on GPSIMD
        for i in range(4):
            nc.vector.tensor_tensor(yca[:, (2 * i) * HW:(2 * i + 1) * HW],
                                    yca[:, (2 * i) * HW:(2 * i + 1) * HW],
                                    yca[:, (2 * i + 1) * HW:(2 * i + 2) * HW],
                                    OP.max)
        nc.vector.tensor_tensor(yca[:, 0:HW], yca[:, 0:HW],
                                yca[:, 2 * HW:3 * HW], OP.max)
        nc.vector.tensor_tensor(yca[:, 4 * HW:5 * HW], yca[:, 4 * HW:5 * HW],
                                yca[:, 6 * HW:7 * HW], OP.max)
        nc.vector.tensor_tensor(yca[:, 0:HW], yca[:, 0:HW],
                                yca[:, 4 * HW:5 * HW], OP.max)
        mxbc = sb2.tile([128, HW], F32, tag="mxbc")
        nc.gpsimd.partition_all_reduce(mxbc[:], yca[:, 0:HW], 128,
                                       bass_isa.ReduceOp.max)
        from concourse import library_config
        nc.gpsimd.load_library(library_config.standard)
        # row 0 of mxbc = per-pixel channel max; to columns via PE transposes
        tid1b = tid1
        mxpx = sb.tile([128, HT], F32, tag="mxpx")
        psmx = psH.tile([128, HT], F32, tag="psh")
        for t in range(HT):
            nc.tensor.transpose(psmx[:, t:t + 1],
                                mxbc[0:1, t * 128:(t + 1) * 128], tid1b[:])
        nc.vector.tensor_copy(mxpx[:], psmx[:])

        # sb broadcast column
        pssb = psH.tile([128, 1], F32, tag="psh")
        nc.tensor.matmul(pssb[:], tonesr[:], sbr[:], start=True, stop=True)
        sbc = sb.tile([128, 1], F32, tag="sbc")
        nc.vector.tensor_copy(sbc[:], pssb[:])

        # spatial 7x7 conv as 6 shifted-column matmuls (2 ch x 3 bands)
        pssp = psH.tile([128, HT], F32, tag="psh")
        mmspecs = []
        for ch, srccol in ((0, avgpx), (1, mxpx)):
            mmspecs.append((ch * 3 + 1, slice(0, 8), srccol[:, 0:8]))
            mmspecs.append((ch * 3 + 2, slice(1, 8), srccol[:, 0:7]))
            mmspecs.append((ch * 3 + 0, slice(0, 7), srccol[:, 1:8]))
        for i, (bi, osl, rhs) in enumerate(mmspecs):
            nc.tensor.matmul(pssp[:, osl], spb[:, bi * 128:(bi + 1) * 128],
                             rhs, start=(i == 0), stop=(i == len(mmspecs) - 1))
        spcol = sb.tile([128, HT], F32, tag="spcol")
        spcolb = sb.tile([128, HT], BF16, tag="spcolb")
        nc.scalar.activation(spcol[:], pssp[:], AF.Sigmoid, bias=sbc[:],
                             scale=1.0)
        # preload Sqrt (for the BN3 affine) while stats3/AG3 are in flight
        nc.scalar.activation(dscr[:, 4:5], spcol[0:1, 0:1], AF.Sqrt,
                             bias=0.0, scale=1.0)
        nc.vector.tensor_copy(spcolb[:], spcol[:])

        # spp = proj * sp (spatial scale, per-partition)
        spp = sb2.tile([128, 8 * 258], BF16, tag="spp")
        for mt in range(HT):
            nc.vector.tensor_scalar(spp[:, mt * 258:mt * 258 + 256],
                                    proj_sb[:, mt * 258:mt * 258 + 256],
                                    spcol[:, mt:mt + 1], None, OP.mult)

        # BN3 stats: sum(sp*proj) and sum((sp*proj)^2) over hw
        pst3a = psH.tile([1, D], F32, tag="psh")
        for mt in range(HT):
            nc.tensor.matmul(pst3a[:], spcolb[:, mt:mt + 1],
                             proj_sb[:, mt * 258:mt * 258 + 256],
                             start=(mt == 0), stop=(mt == 7))
        pst3b = psH.tile([1, D], F32, tag="psh")
        sqs = sb.tile([128, 2 * D], BF16, tag="sqs")
        for mt in range(HT):
            half = (mt % 2) * D
            src = spp[:, mt * 258:mt * 258 + 256]
            nc.vector.scalar_tensor_tensor(
                sqs[:, half:half + D], src, 0.0, src, OP.bypass, OP.mult)
            nc.tensor.matmul(pst3b[:], tonescb[:], sqs[:, half:half + D],
                             start=(mt == 0), stop=(mt == 7))
        stat3l = sb.tile([1, 2 * D], F32, tag="stat3l")
        nc.vector.tensor_copy(stat3l[:, 0:D], pst3a[:])
        nc.vector.tensor_copy(stat3l[:, D:2 * D], pst3b[:])

        # ============================ AG3 (BN3 batch stats)
        bb3i = dram.tile([1, 2 * D], F32, tag="bb3i")
        bb3o = dram.tile([n_cores, 2 * D], F32, tag="bb3o")
        nc.gpsimd.dma_start(bb3i[:], stat3l[:])
        nc.gpsimd.collective_compute(
            "AllGather", OP.bypass, replica_groups=[list(range(n_cores))],
            ins=[bb3i.opt()], outs=[bb3o.opt()])
        gath3 = sb.tile([n_cores, 2 * D], F32, tag="gath3")
        nc.gpsimd.dma_start(gath3[:], bb3o[:])
        pst3g = psH.tile([1, 2 * D], F32, tag="psh")
        nc.tensor.matmul(pst3g[:], tonesc[0:n_cores, :], gath3[:],
                         start=True, stop=True)
        stat3g = sb.tile([1, 2 * D], F32, tag="stat3g")
        nc.vector.tensor_copy(stat3g[:], pst3g[:])

        # BN3 affine in row form (pb cancels through the mean subtraction)
        m3 = sb.tile([1, D], F32, tag="m3")
        v3 = sb.tile([1, D], F32, tag="v3")
        a3r = sb.tile([1, D], F32, tag="a3r")
        c3r = sb.tile([1, D], F32, tag="c3r")
        tmp3 = sb.tile([1, D], F32, tag="tmp3")
        nc.vector.tensor_scalar_mul(m3[:], stat3g[:, 0:D], 1.0 / nb)
        nc.vector.tensor_tensor(tmp3[:], m3[:], m3[:], OP.mult)
        nc.vector.scalar_tensor_tensor(
            v3[:], stat3g[:, D:2 * D], 1.0 / nb, tmp3[:], OP.mult, OP.subtract)
        nc.vector.tensor_scalar_add(v3[:], v3[:], EPS)
        nc.scalar.sqrt(v3[:], v3[:])
        nc.vector.reciprocal(v3[:], v3[:])
        nc.vector.tensor_tensor(a3r[:], g3r[:], v3[:], OP.mult)
        nc.vector.tensor_tensor(tmp3[:], a3r[:], m3[:], OP.mult)
        nc.vector.tensor_tensor(c3r[:], be3r[:], tmp3[:], OP.subtract)

        # broadcast a3/c3 to all partitions (a3 also as bf16 for 2x DVE)
        a3b = sb.tile([128, D], BF16, tag="a3b")
        c3b = sb.tile([128, D], F32, tag="c3b")
        for rowt, dstt in ((a3r, a3b), (c3r, c3b)):
            psx = psH.tile([128, D], F32, tag="psh")
            nc.tensor.matmul(psx[:], tonesr[:], rowt[:], start=True, stop=True)
            nc.vector.tensor_copy(dstt[:], psx[:])

        # final: out = (x + c3) + spp*a3 — split across DVE and GPSIMD
        # (the gpsimd library was switched to `standard` above, so its
        # TensorTensor is usable after partition_all_reduce)
        out_sb = sb2.tile([128, 8 * D], F32, tag="outsb")
        sclb = sb2.tile([128, 8 * D], BF16, tag="sclb")
        for mt in range(HT):
            eng = nc.vector if mt < 5 else nc.gpsimd
            sl = slice(mt * D, (mt + 1) * D)
            ssl = spp[:, mt * 258:mt * 258 + 256]
            eng.tensor_tensor(out_sb[:, sl], xres[:, sl], c3b[:], OP.add)
            eng.tensor_tensor(sclb[:, sl], ssl, a3b[:], OP.mult)
            eng.tensor_tensor(out_sb[:, sl], out_sb[:, sl], sclb[:, sl],
                              OP.add)
            nc.sync.dma_start(
                out_d.ap().rearrange("(t p) d -> p t d", p=128)[:, mt, :],
                out_sb[:, mt * D:(mt + 1) * D])


# ---------------------------------------------------------------- host driver

def stage_shared(inputs):
    """Shared (batch-independent) weights, staged to on-device layouts."""
    w1 = np.asarray(inputs["w1"], np.float32)
    f32 = lambda a: np.ascontiguousarray(np.asarray(a)).astype(np.float32)
    bf = lambda a: np.ascontiguousarray(np.asarray(a)).astype(NP_BF16)
    return {
        "w1t": bf(w1.T),
        "b1r": bf(np.asarray(inputs["b1"]).reshape(1, C)),
        "b1c": f32(np.asarray(inputs["b1"]).reshape(CT, 128).T),
        "g1c": f32(np.asarray(inputs["g1"]).reshape(CT, 128).T),
        "be1c": f32(np.asarray(inputs["be1"]).reshape(CT, 128).T),
        "g2c": f32(np.asarray(inputs["g2"]).reshape(CT, 128).T),
        "be2c": f32(np.asarray(inputs["be2"]).reshape(CT, 128).T),
        "aw1t": bf(np.asarray(inputs["aw1"], np.float32).T),
        "ab1c": f32(np.asarray(inputs["ab1"]).reshape(1, 128).T),
        "aw2t": bf(np.asarray(inputs["aw2"], np.float32).T),
        "ab2r": f32(np.asarray(inputs["ab2"]).reshape(1, 9)),
        "caw1t": bf(np.asarray(inputs["ca_w1"], np.float32).T),
        "caw2t": bf(np.asarray(inputs["ca_w2"], np.float32).T),
        "pwt": bf(np.asarray(inputs["pw"], np.float32).T),
        "g3r": f32(np.asarray(inputs["g3"]).reshape(1, D)),
        "be3r": f32(np.asarray(inputs["be3"]).reshape(1, D)),
        "sbr": f32(np.asarray(inputs["sb"]).reshape(1, 1)),
    }


def shard_inputs(inputs):
    """Full inputs -> per-core in_maps (host-side layout staging only)."""
    x = np.ascontiguousarray(np.asarray(inputs["x"], np.float32))
    bf = lambda a: np.ascontiguousarray(a).astype(NP_BF16)
    shared = stage_shared(inputs)
    in_maps = []
    for i in range(NCORES):
        m = dict(shared)
        m["x"] = np.ascontiguousarray(x[i])
        m["xt"] = bf(x[i].T)
        in_maps.append(m)
    return in_maps


_CACHE = {}


def get_program(sw, sim_gelu_identity=False, n_cores=NCORES, debug=False):
    key = ("sim" if sim_gelu_identity else "hw", n_cores, debug, sw.tobytes())
    if key not in _CACHE:
        _CACHE[key] = build_program(sw, sim_gelu_identity=sim_gelu_identity,
                                    n_cores=n_cores, debug=debug)
    return _CACHE[key]


def run(inputs, trace=False):
    nc = get_program(np.asarray(inputs["sw"], np.float32))
    in_maps = shard_inputs(inputs)
    r = bass_utils.run_bass_kernel_spmd(
        nc, in_maps, core_ids=list(range(NCORES)), trace=trace)
    out = np.stack([r.results[i]["out"] for i in range(NCORES)], axis=0)
    return out.astype(np.float32), r


def kernel(**inputs) -> np.ndarray:
    out, _ = run(inputs, trace=False)
    return out


# revision 23
# speedup vs baseline: 1.0322x; 1.0322x over previous
"""Trainium2 Bass kernel for nn_EnhancedDepthwiseSeparableFFN (v2).

Data-parallel over the batch: 8 samples -> 8 NeuronCores, one sample each.
Cross-core traffic: three tiny AllGathers for the BatchNorm batch statistics.

Changes over the 243us baseline (now ~200-210us; ~115us of the span is
compute, the rest absorbs host-side core-launch stagger at the three
collective barriers):
  - bf16 everywhere on the PE path (FWL weight loads, full-rate stencil
    pieces, cheap LDWEIGHTS) and for the big elementwise tensors (DVE 2x/4x).
  - double-expand: the spatial-major gelu1 output is recomputed as a second
    expand matmul pass (overlapping the AG1 collective) instead of 64 PE
    transposes + PSUM copies.
  - stats ride fused ops: ACT accum (sums) + DVE scalar_tensor_tensor
    accum (sum-of-squares); per-channel max interleaved on DVE.
  - BN1/BN2 rsqrt via the bit-trick + 2 Newton steps on DVE (no ACT Sqrt
    table loads between the Gelu phases); ACT tables are preloaded by dummy
    activations pinned with data deps so every load hides under a
    collective wait; all PSUM->SBUF copies go through DVE (ACT COPY would
    reload the activation table).
  - srow from a host-side tap-indicator matrix (one small matmul);
    b1row via 8 tiny column transposes (no DRAM bounce); L built in 9 wide
    fused DVE ops; stencil PSUM opens pipelined 3 tiles ahead; keep-warm
    matmuls with post-collective deps stop HAM re-throttling.
  - final tail split across DVE and GPSIMD (gpsimd ucode library switched
    to `standard` after partition_all_reduce).
"""
import numpy as np

import concourse.bass as bass
import concourse.bacc as bacc
import concourse.tile as tile
from concourse import mybir, bass_utils, bass_isa

F32 = mybir.dt.float32
BF16 = mybir.dt.bfloat16
U32 = mybir.dt.uint32
AF = mybir.ActivationFunctionType
OP = mybir.AluOpType

NP_BF16 = mybir.dt.np(BF16)

D = 256          # model dim
C = 1024         # expanded channels
H = W = 32
HW = 1024
NCORES = 8
B = 8            # batch
EPS = 1e-5
CT = C // 128    # 8 channel tiles
HT = HW // 128   # 8 spatial tiles
RSQRT_ITERS = 2


# ---------------------------------------------------------------- host consts

def _stencil_masks():
    """(128, 9*384) f32: per q a full [L(-1) | L(0) | L(+1)] contribution
    mask, so L = sum_q kw[q] * M_q builds in 9 wide DVE ops.

    L_delta[k_in, m_out] = kw[q],  q = (dh+1)*3 + (dw+1),
    dh = h_in - h_out = r_in - r_out - 4*delta, dw = w_in - w_out.
    """
    k = np.arange(128)
    m = np.arange(128)
    r_in, w_in = k // 32, k % 32
    r_out, w_out = m // 32, m % 32
    dw = w_in[:, None] - w_out[None, :]
    out = np.zeros((9, 128, 384), np.float32)
    for bi, delta in enumerate((-1, 0, 1)):
        dh = r_in[:, None] - r_out[None, :] - 4 * delta
        for q in range(9):
            dh_q, dw_q = q // 3 - 1, q % 3 - 1
            out[q, :, bi * 128:(bi + 1) * 128] = (
                (dh == dh_q) & (dw == dw_q)).astype(np.float32)
    # delta=-1 only contributes taps q in {6,7,8}; delta=+1 only {0,1,2}
    for q in range(9):
        if q < 6:
            out[q, :, 0:128] = 0.0
        if q > 2:
            out[q, :, 256:384] = 0.0
    return np.concatenate(list(out), axis=1)  # (128, 9*384)


def _spatial_bands(sw):
    """(128, 6*128) f32 lhsT band tiles for the 7x7 conv, cols =
    [ch0 d-1,d0,d+1 | ch1 d-1,d0,d+1]."""
    k = np.arange(128)
    m = np.arange(128)
    r_in, w_in = k // 32, k % 32
    r_out, w_out = m // 32, m % 32
    dw = w_in[:, None] - w_out[None, :]
    wok = np.abs(dw) <= 3
    tiles = []
    for ch in range(2):
        for delta in (-1, 0, 1):
            dh = r_in[:, None] - r_out[None, :] - 4 * delta
            hok = np.abs(dh) <= 3
            t = np.zeros((128, 128), np.float32)
            ok = hok & wok
            t[ok] = sw[0, ch][(dh[ok] + 3, dw[ok] + 3)]
            tiles.append(t)
    return np.concatenate(tiles, axis=1)  # (128, 6*128)


def _tap_counts():
    """(9, 1024) f32: SB9[q, px] = 1 if 3x3 tap q is in-bounds at pixel px.

    srow = kw @ SB9 gives the per-pixel sum of present tap weights.
    """
    px = np.arange(HW)
    h, w = px // W, px % W
    out = np.zeros((9, HW), np.float32)
    for q in range(9):
        dh, dw = q // 3 - 1, q % 3 - 1
        ok = (h + dh >= 0) & (h + dh < H) & (w + dw >= 0) & (w + dw < W)
        out[q] = ok.astype(np.float32)
    return out


# ---------------------------------------------------------------- the program

def build_program(sw, sim_gelu_identity=False, n_cores=NCORES, debug=False):
    gelu_f = AF.Identity if sim_gelu_identity else AF.Gelu

    nc = bacc.Bacc("TRN2", target_bir_lowering=False, debug=False,
                   num_devices=n_cores)

    I = {}
    for name, shape, dt in [
        ("x", [HW, D], F32),
        ("xt", [D, HW], BF16), ("w1t", [D, C], BF16),
        ("b1r", [1, C], BF16),
        ("b1c", [128, CT], F32),
        ("g1c", [128, CT], F32), ("be1c", [128, CT], F32),
        ("g2c", [128, CT], F32), ("be2c", [128, CT], F32),
        ("aw1t", [C, 128], BF16), ("ab1c", [128, 1], F32),
        ("aw2t", [128, 9], BF16), ("ab2r", [1, 9], F32),
        ("caw1t", [C, 64], BF16), ("caw2t", [64, C], BF16),
        ("pwt", [C, D], BF16),
        ("g3r", [1, D], F32), ("be3r", [1, D], F32),
        ("sbr", [1, 1], F32),
    ]:
        I[name] = nc.dram_tensor(name, shape, dt, kind="ExternalInput")
    out_d = nc.dram_tensor("out", [HW, D], F32, kind="ExternalOutput")

    ident = nc.inline_tensor(np.eye(128, dtype=np.float32), name="cident")
    ident1 = nc.inline_tensor(np.ones((1, 1), np.float32), name="cident1")
    ones_row = nc.inline_tensor(np.ones((1, 128), np.float32), name="conesr")
    ones_row_bf = nc.inline_tensor(np.ones((1, 128), NP_BF16), name="conesrb")
    ones_col = nc.inline_tensor(np.ones((128, 1), np.float32), name="conesc")
    ones_col_bf = nc.inline_tensor(np.ones((128, 1), NP_BF16), name="conescb")
    masks_c = nc.inline_tensor(_stencil_masks().astype(NP_BF16), name="cmasks")
    spb_c = nc.inline_tensor(_spatial_bands(sw), name="cspb")
    sb9_c = nc.inline_tensor(_tap_counts().astype(NP_BF16), name="csb9")
    magic_c = nc.inline_tensor(
        np.full((128, 8), 0x5f3759df, np.uint32), name="cmagic")
    one_u32_c = nc.inline_tensor(np.full((128, 8), 1, np.uint32), name="cone32")

    with tile.TileContext(nc) as tc:
        _body(nc, tc, I, out_d, ident, ident1, ones_row, ones_row_bf,
              ones_col, ones_col_bf, masks_c, spb_c, sb9_c, magic_c,
              one_u32_c, gelu_f, n_cores)
    nc.compile()
    return nc


def _rsqrt_cols(nc, sb, magic, one32, v, n, tag):
    """DVE-only rsqrt of v (128, n) f32 (in place OK) -> returns (128, n).

    Bit-trick initial guess + 2 Newton steps; rel err ~5e-6.
    """
    y = sb.tile([128, n], F32, tag=tag + "y")
    t = sb.tile([128, n], F32, tag=tag + "t")
    yu = y[:].bitcast(U32)
    nc.vector.tensor_tensor(yu, v[:].bitcast(U32), one32[:, 0:n],
                            OP.logical_shift_right)
    nc.vector.tensor_tensor(yu, magic[:, 0:n], yu, OP.subtract)
    for _ in range(RSQRT_ITERS):
        nc.vector.tensor_tensor(t[:], y[:], y[:], OP.mult)
        nc.vector.tensor_tensor(t[:], t[:], v[:], OP.mult)
        nc.vector.tensor_scalar(t[:], t[:], -0.5, 1.5, OP.mult, OP.add)
        nc.vector.tensor_tensor(y[:], y[:], t[:], OP.mult)
    return y


def _body(nc, tc, I, out_d, ident, ident1, ones_row, ones_row_bf, ones_col,
          ones_col_bf, masks_c, spb_c, sb9_c, magic_c, one_u32_c, gelu_f,
          n_cores=NCORES):
    nb = n_cores * HW

    with tc.tile_pool(name="sb", bufs=1) as sb, \
         tc.tile_pool(name="sb2", bufs=1) as sb2, \
         tc.tile_pool(name="psb", bufs=3, space="PSUM") as psB, \
         tc.tile_pool(name="psh", bufs=2, space="PSUM") as psH, \
         tc.tile_pool(name="dram", bufs=6, space="DRAM") as dram:

        def load(name, shape, ap_in, dt=F32, pool=sb, view=None):
            t = pool.tile(shape, dt, tag=name)
            dst = t[:] if view is None else t[:].rearrange(*view[0], **view[1])
            nc.sync.dma_start(dst, ap_in)
            return t

        # ---------------- persistent SBUF tensors + loads
        # (ordered by first use: phase-1 operands first so the expand
        # matmuls are not stuck behind late-needed bulk transfers)
        tid1 = load("tid1", [1, 1], ident1.ap())
        b1c = load("b1c", [128, CT], I["b1c"].ap())
        xt_sb = sb.tile([128, 2 * HW], BF16, tag="xt_sb")
        w1t_sb = sb.tile([128, 2 * C], BF16, tag="w1t_sb")
        for k in range(2):
            nc.sync.dma_start(
                xt_sb[:, k * HW:(k + 1) * HW],
                I["xt"].ap().rearrange("(k p) n -> p k n", p=128)[:, k, :])
            nc.sync.dma_start(
                w1t_sb[:, k * C:(k + 1) * C],
                I["w1t"].ap().rearrange("(k p) n -> p k n", p=128)[:, k, :])
        tonesrb = load("tonesrb", [1, 128], ones_row_bf.ap(), dt=BF16)
        b1rb = load("b1rb", [1, C], I["b1r"].ap(), dt=BF16)
        magic = load("magic", [128, 8], magic_c.ap(), dt=U32)
        one32 = load("one32", [128, 8], one_u32_c.ap(), dt=U32)
        g1c = load("g1c", [128, CT], I["g1c"].ap())
        be1c = load("be1c", [128, CT], I["be1c"].ap())
        ab1c = load("ab1c", [128, 1], I["ab1c"].ap())
        aw1t_sb = load("aw1t_sb", [128, 8 * 128],
                       I["aw1t"].ap().rearrange("(k p) n -> p k n", p=128),
                       dt=BF16, view=(("p (k n) -> p k n",), dict(k=8)))
        aw2t_sb = load("aw2t_sb", [128, 9], I["aw2t"].ap(), dt=BF16)
        ab2r = load("ab2r", [1, 9], I["ab2r"].ap())
        tid = load("tid", [128, 128], ident.ap())
        tonesr = load("tonesr", [1, 128], ones_row.ap())
        masks = load("masks", [128, 9 * 384], masks_c.ap(), dt=BF16)
        sb9 = load("sb9", [9, HW], sb9_c.ap(), dt=BF16)
        g2c = load("g2c", [128, CT], I["g2c"].ap())
        be2c = load("be2c", [128, CT], I["be2c"].ap())
        caw1t_sb = load("caw1t_sb", [128, 8 * 64],
                        I["caw1t"].ap().rearrange("(k p) n -> p k n", p=128),
                        dt=BF16, view=(("p (k n) -> p k n",), dict(k=8)))
        caw2t_sb = load("caw2t_sb", [64, C], I["caw2t"].ap(), dt=BF16)
        paug = sb.tile([128, 8 * 260], BF16, tag="paug")
        nc.sync.dma_start(
            paug[:].rearrange("p (k n) -> p k n", n=260)[:, :, 0:D],
            I["pwt"].ap().rearrange("(k p) n -> p k n", p=128))
        nc.vector.memset(
            paug[:].rearrange("p (k n) -> p k n", n=260)[:, :, D:D + 1],
            1.0 / C)
        nc.vector.memset(
            paug[:].rearrange("p (k n) -> p k n", n=260)[:, :, D + 1:D + 2],
            0.0)
        tonesc = load("tonesc", [128, 1], ones_col.ap())
        tonescb = load("tonescb", [128, 1], ones_col_bf.ap(), dt=BF16)
        spb = load("spb", [128, 6 * 128], spb_c.ap())
        g3r = load("g3r", [1, D], I["g3r"].ap())
        be3r = load("be3r", [1, D], I["be3r"].ap())
        sbr = load("sbr", [1, 1], I["sbr"].ap())
        xres = load("xres", [128, 8 * D],
                    I["x"].ap().rearrange("(t p) d -> p t d", p=128),
                    view=(("p (t d) -> p t d",), dict(t=8)))

        # big working tensors (bf16)
        yg_sp = sb2.tile([128, HT * C], BF16, tag="ygsp")   # gelu1, spatial-major
        g2o = sb2.tile([128, CT * HW], BF16, tag="g2o")     # gelu2, ch-major
        yca = sb2.tile([128, CT * HW], BF16, tag="yca")     # ch-att out
        ygscr = [sb.tile([128, HW], BF16, tag=f"ygscr{i}", name=f"ygscr{i}")
                 for i in range(2)]
        sqscr = [sb.tile([128, HW], BF16, tag=f"sqscr{i}", name=f"sqscr{i}")
                 for i in range(2)]
        stat1l = sb.tile([128, 16], F32, tag="stat1l")
        stat1g = sb.tile([128, 16], F32, tag="stat1g")
        stat2l = sb.tile([128, 16], F32, tag="stat2l")
        stat2g = sb.tile([128, 16], F32, tag="stat2g")
        dscr = sb.tile([1, 8], F32, tag="dscr")             # ACT table preload dst

        # table preload: gelu load hides under the input DMAs
        nc.scalar.activation(dscr[:, 0:1], tid1[:], gelu_f, bias=0.0,
                             scale=1.0)

        # ============================ PHASE 1a: expand (ch-major) + stats1
        for m in range(CT):
            ps = psB.tile([128, HW], F32, tag="psb")
            for k in range(2):      # k outer: one LDWEIGHTS per (m, k)
                for h in range(2):
                    nc.tensor.matmul(
                        ps[:, h * 512:(h + 1) * 512],
                        w1t_sb[:, k * C + m * 128: k * C + (m + 1) * 128],
                        xt_sb[:, k * HW + h * 512: k * HW + (h + 1) * 512],
                        start=(k == 0), stop=(k == 1))
            yscr = ygscr[m % 2]
            nc.scalar.activation(
                yscr[:], ps[:], gelu_f, bias=b1c[:, m:m + 1], scale=1.0,
                accum_out=stat1l[:, m:m + 1])
            nc.vector.scalar_tensor_tensor(
                sqscr[m % 2][:], yscr[:], 0.0, yscr[:], OP.bypass, OP.mult,
                accum_out=stat1l[:, 8 + m:9 + m])

        # ============================ AG1 (BN1 batch stats)
        bb1i = dram.tile([128, 16], F32, tag="bb1i")
        bb1o = dram.tile([n_cores * 128, 16], F32, tag="bb1o")
        nc.gpsimd.dma_start(bb1i[:], stat1l[:])
        nc.gpsimd.collective_compute(
            "AllGather", OP.bypass, replica_groups=[list(range(n_cores))],
            ins=[bb1i.opt()], outs=[bb1o.opt()])

        # ============================ PHASE 1b: expand again, spatial-major
        # (runs on PE/ACT while the AG1 collective is in flight)
        for t in range(HT):
            ps2 = psB.tile([128, HW], F32, tag="psb")
            for k in range(2):      # k outer: one LDWEIGHTS per (t, k)
                for g in range(2):
                    nc.tensor.matmul(
                        ps2[:, g * 512:(g + 1) * 512],
                        xt_sb[:, k * HW + t * 128: k * HW + (t + 1) * 128],
                        w1t_sb[:, k * C + g * 512: k * C + (g + 1) * 512],
                        start=(k == 0), stop=False)
            for g in range(2):
                nc.tensor.matmul(
                    ps2[:, g * 512:(g + 1) * 512],
                    tonesrb[:],
                    b1rb[:, g * 512:(g + 1) * 512],
                    start=False, stop=True)
            nc.scalar.activation(
                yg_sp[:, t * C:(t + 1) * C], ps2[:], gelu_f,
                bias=0.0, scale=1.0)
        # preload the Exp table right after the last sp-pass gelu (the
        # data dep pins it there; the load hides under the AG1 flight)
        nc.scalar.activation(dscr[:, 1:2], yg_sp[0:1, HT * C - 1:HT * C],
                             AF.Exp, bias=0.0, scale=1.0)

        # gather AG1 result + local combine
        gath1 = sb.tile([128, n_cores * 16], F32, tag="gath1")
        nc.gpsimd.dma_start(
            gath1[:].rearrange("p (r f) -> p r f", f=16),
            bb1o[:].rearrange("(r p) f -> p r f", p=128))
        nc.vector.tensor_reduce(
            stat1g[:], gath1[:].rearrange("p (r f) -> p f r", f=16),
            mybir.AxisListType.X, OP.add)

        # ============================ PHASE 3: BN1 affine + kw + L build
        def bn_affine(statg, gcol, becol, tagp):
            """-> (a, bn) per-channel scale/shift columns (128, CT)."""
            mns = sb.tile([128, CT], F32, tag=tagp + "m")
            var = sb.tile([128, CT], F32, tag=tagp + "v")
            a = sb.tile([128, CT], F32, tag=tagp + "a")
            bn = sb.tile([128, CT], F32, tag=tagp + "b")
            nc.vector.tensor_scalar_mul(mns[:], statg[:, 0:8], 1.0 / nb)
            nc.vector.tensor_tensor(var[:], mns[:], mns[:], OP.mult)
            nc.vector.scalar_tensor_tensor(
                var[:], statg[:, 8:16], 1.0 / nb, var[:], OP.mult, OP.subtract)
            nc.vector.tensor_scalar_add(var[:], var[:], EPS)
            rs = _rsqrt_cols(nc, sb, magic, one32, var, CT, tagp + "r")
            nc.vector.tensor_tensor(a[:], gcol[:], rs[:], OP.mult)
            nc.vector.tensor_tensor(bn[:], mns[:], a[:], OP.mult)
            nc.vector.tensor_tensor(bn[:], becol[:], bn[:], OP.subtract)
            return a, bn

        a1, b1n = bn_affine(stat1g, g1c, be1c, "s1")
        inva1 = sb.tile([128, CT], F32, tag="inva1")
        bpre = sb.tile([128, CT], F32, tag="bpre")
        nc.vector.reciprocal(inva1[:], a1[:])
        nc.vector.tensor_tensor(bpre[:], b1n[:], inva1[:], OP.mult)
        # b' row (1, C) via 8 tiny column transposes (no DRAM bounce)
        b1rowb = sb.tile([1, C], BF16, tag="b1rowb")
        for half in range(2):
            psb1 = psH.tile([1, 512], F32, tag="psh")
            for j in range(4):
                c = half * 4 + j
                nc.tensor.transpose(psb1[:, j * 128:(j + 1) * 128],
                                    bpre[:, c:c + 1], tid[:])
            nc.vector.tensor_copy(b1rowb[:, half * 512:(half + 1) * 512], psb1[:])

        # gap (local, normalized) -> kw
        gapn = sb.tile([128, CT], F32, tag="gapn")
        gapb = sb.tile([128, CT], BF16, tag="gapb")
        nc.vector.scalar_tensor_tensor(
            gapn[:], stat1l[:, 0:8], 1.0 / HW, a1[:], OP.mult, OP.mult)
        nc.vector.tensor_tensor(gapn[:], gapn[:], b1n[:], OP.add)
        nc.vector.tensor_copy(gapb[:], gapn[:])

        # PE keep-warm: dummy matmuls with post-AG1 deps so HAM does not
        # re-throttle during the mostly-serial kw chain
        for i in range(4):
            psw = psH.tile([1, 512], F32, tag="psh")
            nc.tensor.matmul(psw[:], gapb[:, 0:1],
                             xt_sb[:, i * 512:(i + 1) * 512],
                             start=True, stop=True)
        ph1 = psH.tile([128, 1], F32, tag="psh")
        for k in range(CT):
            nc.tensor.matmul(ph1[:], aw1t_sb[:, k * 128:(k + 1) * 128],
                             gapb[:, k:k + 1], start=(k == 0), stop=(k == 7))
        h1 = sb.tile([128, 1], BF16, tag="h1")
        nc.vector.tensor_scalar(h1[:], ph1[:], ab1c[:], 0.0, OP.add, OP.max)
        ps9 = psH.tile([1, 9], F32, tag="psh")
        nc.tensor.matmul(ps9[:], h1[:], aw2t_sb[:], start=True, stop=True)
        v9 = sb.tile([1, 9], F32, tag="v9")
        nc.vector.tensor_tensor(v9[:], ps9[:], ab2r[:], OP.add)
        mx9 = sb.tile([1, 1], F32, tag="mx9")
        nc.vector.tensor_reduce(mx9[:], v9[:], mybir.AxisListType.X, OP.max)
        nc.vector.tensor_scalar(v9[:], v9[:], mx9[:], None, OP.subtract)
        e9 = sb.tile([1, 9], F32, tag="e9")
        se = sb.tile([1, 1], F32, tag="se")
        nc.scalar.activation(e9[:], v9[:], AF.Exp, bias=0.0, scale=1.0,
                             accum_out=se[:])
        # re-preload Gelu for phase 4 (dep on the exp output pins it after
        # the real Exp; the load hides under the kw/L-build chain)
        nc.scalar.activation(dscr[:, 2:3], e9[:, 0:1], gelu_f, bias=0.0,
                             scale=1.0)
        rse = sb.tile([1, 1], F32, tag="rse")
        nc.vector.reciprocal(rse[:], se[:])
        kw9 = sb.tile([1, 9], F32, tag="kw9")
        nc.vector.tensor_scalar(kw9[:], e9[:], rse[:], None, OP.mult)
        # broadcast kw to all partitions (for the L build scalars)
        pskb = psH.tile([128, 9], F32, tag="psh")
        nc.tensor.matmul(pskb[:], tonesr[:], kw9[:], start=True, stop=True)
        kwb = sb.tile([128, 9], F32, tag="kwb")
        nc.vector.tensor_copy(kwb[:], pskb[:])
        kwbb = sb.tile([128, 1], BF16, tag="kwbb")
        nc.vector.tensor_copy(kwbb[:], kwb[:, 0:1])
        for i in range(4):
            psw = psH.tile([1, 512], F32, tag="psh")
            nc.tensor.matmul(psw[:], kwbb[:],
                             xt_sb[:, i * 512:(i + 1) * 512],
                             start=True, stop=True)
        # kw as a column (9, 1) for the srow matmul
        pskc = psH.tile([9, 1], F32, tag="psh")
        nc.tensor.transpose(pskc[:], kw9[:], tid1[:])
        kwcol = sb.tile([9, 1], BF16, tag="kwcol")
        nc.vector.tensor_copy(kwcol[:], pskc[:])

        # L band tiles, concatenated [L(-1) | L(0) | L(+1)]: 9 wide DVE ops
        L = sb.tile([128, 3 * 128], BF16, tag="L")
        nc.vector.tensor_scalar(L[:], masks[:, 0:384], kwb[:, 0:1],
                                None, OP.mult)
        for q in range(1, 9):
            nc.vector.scalar_tensor_tensor(
                L[:], masks[:, q * 384:(q + 1) * 384], kwb[:, q:q + 1],
                L[:], OP.mult, OP.add)

        # srow = kw @ SB9 (per-pixel sum of present taps)
        srowb = sb.tile([1, HW], BF16, tag="srowb")
        for h in range(2):
            pss = psH.tile([1, 512], F32, tag="psh")
            nc.tensor.matmul(pss[:], kwcol[:],
                             sb9[:, h * 512:(h + 1) * 512],
                             start=True, stop=True)
            nc.vector.tensor_copy(srowb[:, h * 512:(h + 1) * 512], pss[:])

        # ============================ PHASE 4: stencil + gelu2 + stats2
        # rank-1 opens are emitted up to 3 tiles ahead so the PE can run
        # them (and warm up) while the DVE is still building L
        mxc = sb.tile([128, CT], F32, tag="mxc")
        psz_q = {}

        def open_psz(c):
            psz = psB.tile([128, HW], F32, tag="psb")
            for h in range(2):
                nc.tensor.matmul(psz[:, h * 512:(h + 1) * 512],
                                 b1rowb[:, c * 128:(c + 1) * 128],
                                 srowb[:, h * 512:(h + 1) * 512],
                                 start=True, stop=False)
            psz_q[c] = psz

        for c in range(3):
            open_psz(c)
        for c in range(CT):
            psz = psz_q.pop(c)
            for t_in in range(HT):
                lo = max(0, (t_in - 1) * 128)
                hi = min(HW, (t_in + 2) * 128)
                roff = 128 + (lo - t_in * 128)
                if lo < 512 < hi:
                    pieces = [(lo, 512), (512, hi)]
                else:
                    pieces = [(lo, hi)]
                for (a, b) in pieces:
                    ra = roff + (a - lo)
                    last_bank0 = (a < 512) and (t_in == 4)
                    last_bank1 = (a >= 512) and (t_in == 7)
                    nc.tensor.matmul(
                        psz[:, a:b],
                        yg_sp[:, t_in * C + c * 128: t_in * C + (c + 1) * 128],
                        L[:, ra:ra + (b - a)],
                        start=False, stop=(last_bank0 or last_bank1))
            nc.scalar.activation(
                g2o[:, c * HW:(c + 1) * HW], psz[:], gelu_f,
                bias=0.0, scale=a1[:, c:c + 1],
                accum_out=stat2l[:, c:c + 1])
            if c + 3 < CT:
                open_psz(c + 3)
            srcg2 = g2o[:, c * HW:(c + 1) * HW]
            nc.vector.scalar_tensor_tensor(
                sqscr[c % 2][:], srcg2, 0.0, srcg2, OP.bypass, OP.mult,
                accum_out=stat2l[:, 8 + c:9 + c])

        # ============================ AG2 (BN2 batch stats)
        bb2i = dram.tile([128, 16], F32, tag="bb2i")
        bb2o = dram.tile([n_cores * 128, 16], F32, tag="bb2o")
        nc.gpsimd.dma_start(bb2i[:], stat2l[:])
        nc.gpsimd.collective_compute(
            "AllGather", OP.bypass, replica_groups=[list(range(n_cores))],
            ins=[bb2i.opt()], outs=[bb2o.opt()])
        # per-channel max over HW: DVE is idle while AG2 is in flight
        for c in range(CT):
            nc.vector.tensor_reduce(mxc[:, c:c + 1],
                                    g2o[:, c * HW:(c + 1) * HW],
                                    mybir.AxisListType.X, OP.max)
        # preload Sigmoid while AG2 is in flight (dep pins it after the
        # last stencil gelu)
        nc.scalar.activation(dscr[:, 3:4], g2o[0:1, CT * HW - 1:CT * HW],
                             AF.Sigmoid, bias=0.0, scale=1.0)
        gath2 = sb.tile([128, n_cores * 16], F32, tag="gath2")
        nc.gpsimd.dma_start(
            gath2[:].rearrange("p (r f) -> p r f", f=16),
            bb2o[:].rearrange("(r p) f -> p r f", p=128))
        nc.vector.tensor_reduce(
            stat2g[:], gath2[:].rearrange("p (r f) -> p f r", f=16),
            mybir.AxisListType.X, OP.add)

        # ============================ PHASE 6: BN2 + channel attention
        a2, b2n = bn_affine(stat2g, g2c, be2c, "s2")
        amx = sb.tile([128, 2 * CT], F32, tag="amx")
        amxb = sb.tile([128, 2 * CT], BF16, tag="amxb")
        nc.vector.scalar_tensor_tensor(
            amx[:, 0:8], stat2l[:, 0:8], 1.0 / HW, a2[:], OP.mult, OP.mult)
        nc.vector.tensor_tensor(amx[:, 0:8], amx[:, 0:8], b2n[:], OP.add)
        nc.vector.tensor_tensor(amx[:, 8:16], mxc[:], a2[:], OP.mult)
        nc.vector.tensor_tensor(amx[:, 8:16], amx[:, 8:16], b2n[:], OP.add)
        nc.vector.tensor_copy(amxb[:], amx[:])

        for i in range(4):
            psw = psH.tile([1, 512], F32, tag="psh")
            nc.tensor.matmul(psw[:], amxb[:, 0:1],
                             xt_sb[:, i * 512:(i + 1) * 512],
                             start=True, stop=True)
        psf = psH.tile([64, 2], F32, tag="psh")
        for k in range(CT):
            nc.tensor.matmul(psf[:], caw1t_sb[:, k * 64:(k + 1) * 64],
                             amxb[:, k:k + 9:8], start=(k == 0), stop=(k == 7))
        hsum = sb.tile([64, 1], BF16, tag="hsum")
        hp = sb.tile([64, 2], F32, tag="hp")
        nc.vector.tensor_scalar(hp[:], psf[:], 0.0, None, OP.max)
        nc.vector.tensor_tensor(hsum[:], hp[:, 0:1], hp[:, 1:2], OP.add)

        psc = psH.tile([128, CT], F32, tag="psh")
        for c in range(CT):
            nc.tensor.matmul(psc[:, c:c + 1], caw2t_sb[:, c * 128:(c + 1) * 128],
                             hsum[:], start=True, stop=True)
        scol = sb.tile([128, CT], F32, tag="scol")
        nc.scalar.activation(scol[:], psc[:], AF.Sigmoid, bias=0.0, scale=1.0)

        sprime = sb.tile([128, CT], F32, tag="sprime")
        b2s = sb.tile([128, CT], F32, tag="b2s")
        nc.vector.tensor_tensor(sprime[:], scol[:], a2[:], OP.mult)
        nc.vector.tensor_tensor(b2s[:], scol[:], b2n[:], OP.mult)

        # y_ca = sprime*g2o + b2s: used as the projection lhsT (it absorbs
        # both the pws scaling and the b2s-rank-1 term) AND for the
        # channel-max tree
        for c in range(CT):
            nc.vector.tensor_scalar(yca[:, c * HW:(c + 1) * HW],
                                    g2o[:, c * HW:(c + 1) * HW],
                                    sprime[:, c:c + 1], b2s[:, c:c + 1],
                                    OP.mult, OP.add)

        # projection -> proj_sb (spatial-major (hw, d)):
        # psp[px, d] = sum_ch yca[ch, px] * paug[ch, d]
        proj_sb = sb2.tile([128, 8 * 258], BF16, tag="proj_sb")
        avgpx = sb.tile([128, HT], F32, tag="avgpx")
        for mt in range(HT):
            psp = psH.tile([128, 258], F32, tag="psh")
            for c in range(CT):
                nc.tensor.matmul(psp[:],
                                 yca[:, c * HW + mt * 128: c * HW + (mt + 1) * 128],
                                 paug[:, c * 260:c * 260 + 258],
                                 start=(c == 0), stop=(c == 7))
            dst = proj_sb[:, mt * 258:mt * 258 + 258]
            nc.vector.tensor_copy(dst, psp[:])
            nc.vector.tensor_copy(avgpx[:, mt:mt + 1], psp[:, D:D + 1])

        # channel max (per pixel): pairwise tree into a separate buffer so
        # the projection matmuls (which read yca) run concurrently
        yct = sb2.tile([128, 4 * HW], BF16, tag="yct")
        for i in range(4):
            nc.vector.tensor_tensor(yct[:, i * HW:(i + 1) * HW],
                                    yca[:, (2 * i) * HW:(2 * i + 1) * HW],
                                    yca[:, (2 * i + 1) * HW:(2 * i + 2) * HW],
                                    OP.max)
        nc.vector.tensor_tensor(yct[:, 0:HW], yct[:, 0:HW],
                                yct[:, HW:2 * HW], OP.max)
        nc.vector.tensor_tensor(yct[:, 2 * HW:3 * HW], yct[:, 2 * HW:3 * HW],
                                yct[:, 3 * HW:4 * HW], OP.max)
        nc.vector.tensor_tensor(yct[:, 0:HW], yct[:, 0:HW],
                                yct[:, 2 * HW:3 * HW], OP.max)
        mxbc = sb2.tile([128, HW], F32, tag="mxbc")
        nc.gpsimd.partition_all_reduce(mxbc[:], yct[:, 0:HW], 128,
                                       bass_isa.ReduceOp.max)
        from concourse import library_config
        nc.gpsimd.load_library(library_config.standard)
        # row 0 of mxbc = per-pixel channel max; to columns via PE transposes
        tid1b = tid1
        mxpx = sb.tile([128, HT], F32, tag="mxpx")
        psmx = psH.tile([128, HT], F32, tag="psh")
        for t in range(HT):
            nc.tensor.transpose(psmx[:, t:t + 1],
                                mxbc[0:1, t * 128:(t + 1) * 128], tid1b[:])
        nc.vector.tensor_copy(mxpx[:], psmx[:])

        # sb broadcast column
        pssb = psH.tile([128, 1], F32, tag="psh")
        nc.tensor.matmul(pssb[:], tonesr[:], sbr[:], start=True, stop=True)
        sbc = sb.tile([128, 1], F32, tag="sbc")
        nc.vector.tensor_copy(sbc[:], pssb[:])

        # spatial 7x7 conv as 6 shifted-column matmuls (2 ch x 3 bands)
        pssp = psH.tile([128, HT], F32, tag="psh")
        mmspecs = []
        for ch, srccol in ((0, avgpx), (1, mxpx)):
            mmspecs.append((ch * 3 + 1, slice(0, 8), srccol[:, 0:8]))
            mmspecs.append((ch * 3 + 2, slice(1, 8), srccol[:, 0:7]))
            mmspecs.append((ch * 3 + 0, slice(0, 7), srccol[:, 1:8]))
        for i, (bi, osl, rhs) in enumerate(mmspecs):
            nc.tensor.matmul(pssp[:, osl], spb[:, bi * 128:(bi + 1) * 128],
                             rhs, start=(i == 0), stop=(i == len(mmspecs) - 1))
        spcol = sb.tile([128, HT], F32, tag="spcol")
        spcolb = sb.tile([128, HT], BF16, tag="spcolb")
        nc.scalar.activation(spcol[:], pssp[:], AF.Sigmoid, bias=sbc[:],
                             scale=1.0)
        # preload Sqrt (for the BN3 affine) while stats3/AG3 are in flight
        nc.scalar.activation(dscr[:, 4:5], spcol[0:1, 0:1], AF.Sqrt,
                             bias=0.0, scale=1.0)
        nc.vector.tensor_copy(spcolb[:], spcol[:])

        # spp = proj * sp (spatial scale, per-partition)
        spp = sb2.tile([128, 8 * 258], BF16, tag="spp")
        for mt in range(HT):
            nc.vector.tensor_scalar(spp[:, mt * 258:mt * 258 + 256],
                                    proj_sb[:, mt * 258:mt * 258 + 256],
                                    spcol[:, mt:mt + 1], None, OP.mult)

        # BN3 stats: sum(sp*proj) and sum((sp*proj)^2) over hw
        pst3a = psH.tile([1, D], F32, tag="psh")
        for mt in range(HT):
            nc.tensor.matmul(pst3a[:], spcolb[:, mt:mt + 1],
                             proj_sb[:, mt * 258:mt * 258 + 256],
                             start=(mt == 0), stop=(mt == 7))
        pst3b = psH.tile([1, D], F32, tag="psh")
        sqs = sb.tile([128, 2 * D], BF16, tag="sqs")
        for mt in range(HT):
            half = (mt % 2) * D
            src = spp[:, mt * 258:mt * 258 + 256]
            nc.vector.scalar_tensor_tensor(
                sqs[:, half:half + D], src, 0.0, src, OP.bypass, OP.mult)
            nc.tensor.matmul(pst3b[:], tonescb[:], sqs[:, half:half + D],
                             start=(mt == 0), stop=(mt == 7))
        stat3l = sb.tile([1, 2 * D], F32, tag="stat3l")
        nc.vector.tensor_copy(stat3l[:, 0:D], pst3a[:])
        nc.vector.tensor_copy(stat3l[:, D:2 * D], pst3b[:])

        # ============================ AG3 (BN3 batch stats)
        bb3i = dram.tile([1, 2 * D], F32, tag="bb3i")
        bb3o = dram.tile([n_cores, 2 * D], F32, tag="bb3o")
        nc.gpsimd.dma_start(bb3i[:], stat3l[:])
        nc.gpsimd.collective_compute(
            "AllGather", OP.bypass, replica_groups=[list(range(n_cores))],
            ins=[bb3i.opt()], outs=[bb3o.opt()])
        gath3 = sb.tile([n_cores, 2 * D], F32, tag="gath3")
        nc.gpsimd.dma_start(gath3[:], bb3o[:])
        pst3g = psH.tile([1, 2 * D], F32, tag="psh")
        nc.tensor.matmul(pst3g[:], tonesc[0:n_cores, :], gath3[:],
                         start=True, stop=True)
        stat3g = sb.tile([1, 2 * D], F32, tag="stat3g")
        nc.vector.tensor_copy(stat3g[:], pst3g[:])

        # BN3 affine in row form (pb cancels through the mean subtraction)
        m3 = sb.tile([1, D], F32, tag="m3")
        v3 = sb.tile([1, D], F32, tag="v3")
        a3r = sb.tile([1, D], F32, tag="a3r")
        c3r = sb.tile([1, D], F32, tag="c3r")
        tmp3 = sb.tile([1, D], F32, tag="tmp3")
        nc.vector.tensor_scalar_mul(m3[:], stat3g[:, 0:D], 1.0 / nb)
        nc.vector.tensor_tensor(tmp3[:], m3[:], m3[:], OP.mult)
        nc.vector.scalar_tensor_tensor(
            v3[:], stat3g[:, D:2 * D], 1.0 / nb, tmp3[:], OP.mult, OP.subtract)
        nc.vector.tensor_scalar_add(v3[:], v3[:], EPS)
        nc.scalar.sqrt(v3[:], v3[:])
        nc.vector.reciprocal(v3[:], v3[:])
        nc.vector.tensor_tensor(a3r[:], g3r[:], v3[:], OP.mult)
        nc.vector.tensor_tensor(tmp3[:], a3r[:], m3[:], OP.mult)
        nc.vector.tensor_tensor(c3r[:], be3r[:], tmp3[:], OP.subtract)

        # broadcast a3/c3 to all partitions (a3 also as bf16 for 2x DVE)
        a3b = sb.tile([128, D], BF16, tag="a3b")
        c3b = sb.tile([128, D], F32, tag="c3b")
        for rowt, dstt in ((a3r, a3b), (c3r, c3b)):
            psx = psH.tile([128, D], F32, tag="psh")
            nc.tensor.matmul(psx[:], tonesr[:], rowt[:], start=True, stop=True)
            nc.vector.tensor_copy(dstt[:], psx[:])

        # final: out = (x + c3) + spp*a3 — split across DVE and GPSIMD
        # (the gpsimd library was switched to `standard` above, so its
        # TensorTensor is usable after partition_all_reduce)
        out_sb = sb2.tile([128, 8 * D], F32, tag="outsb")
        sclb = sb2.tile([128, 8 * D], BF16, tag="sclb")
        for mt in range(HT):
            eng = nc.vector if mt < 5 else nc.gpsimd
            sl = slice(mt * D, (mt + 1) * D)
            ssl = spp[:, mt * 258:mt * 258 + 256]
            eng.tensor_tensor(out_sb[:, sl], xres[:, sl], c3b[:], OP.add)
            eng.tensor_tensor(sclb[:, sl], ssl, a3b[:], OP.mult)
            eng.tensor_tensor(out_sb[:, sl], out_sb[:, sl], sclb[:, sl],
                              OP.add)
            nc.sync.dma_start(
                out_d.ap().rearrange("(t p) d -> p t d", p=128)[:, mt, :],
                out_sb[:, mt * D:(mt + 1) * D])


# ---------------------------------------------------------------- host driver

def stage_shared(inputs):
    """Shared (batch-independent) weights, staged to on-device layouts."""
    w1 = np.asarray(inputs["w1"], np.float32)
    f32 = lambda a: np.ascontiguousarray(np.asarray(a)).astype(np.float32)
    bf = lambda a: np.ascontiguousarray(np.asarray(a)).astype(NP_BF16)
    return {
        "w1t": bf(w1.T),
        "b1r": bf(np.asarray(inputs["b1"]).reshape(1, C)),
        "b1c": f32(np.asarray(inputs["b1"]).reshape(CT, 128).T),
        "g1c": f32(np.asarray(inputs["g1"]).reshape(CT, 128).T),
        "be1c": f32(np.asarray(inputs["be1"]).reshape(CT, 128).T),
        "g2c": f32(np.asarray(inputs["g2"]).reshape(CT, 128).T),
        "be2c": f32(np.asarray(inputs["be2"]).reshape(CT, 128).T),
        "aw1t": bf(np.asarray(inputs["aw1"], np.float32).T),
        "ab1c": f32(np.asarray(inputs["ab1"]).reshape(1, 128).T),
        "aw2t": bf(np.asarray(inputs["aw2"], np.float32).T),
        "ab2r": f32(np.asarray(inputs["ab2"]).reshape(1, 9)),
        "caw1t": bf(np.asarray(inputs["ca_w1"], np.float32).T),
        "caw2t": bf(np.asarray(inputs["ca_w2"], np.float32).T),
        "pwt": bf(np.asarray(inputs["pw"], np.float32).T),
        "g3r": f32(np.asarray(inputs["g3"]).reshape(1, D)),
        "be3r": f32(np.asarray(inputs["be3"]).reshape(1, D)),
        "sbr": f32(np.asarray(inputs["sb"]).reshape(1, 1)),
    }


def shard_inputs(inputs):
    """Full inputs -> per-core in_maps (host-side layout staging only)."""
    x = np.ascontiguousarray(np.asarray(inputs["x"], np.float32))
    bf = lambda a: np.ascontiguousarray(a).astype(NP_BF16)
    shared = stage_shared(inputs)
    in_maps = []
    for i in range(NCORES):
        m = dict(shared)
        m["x"] = np.ascontiguousarray(x[i])
        m["xt"] = bf(x[i].T)
        in_maps.append(m)
    return in_maps


_CACHE = {}


def get_program(sw, sim_gelu_identity=False, n_cores=NCORES, debug=False):
    key = ("sim" if sim_gelu_identity else "hw", n_cores, debug, sw.tobytes())
    if key not in _CACHE:
        _CACHE[key] = build_program(sw, sim_gelu_identity=sim_gelu_identity,
                                    n_cores=n_cores, debug=debug)
    return _CACHE[key]


def run(inputs, trace=False):
    nc = get_program(np.asarray(inputs["sw"], np.float32))
    in_maps = shard_inputs(inputs)
    r = bass_utils.run_bass_kernel_spmd(
        nc, in_maps, core_ids=list(range(NCORES)), trace=trace)
    out = np.stack([r.results[i]["out"] for i in range(NCORES)], axis=0)
    return out.astype(np.float32), r


def kernel(**inputs) -> np.ndarray:
    out, _ = run(inputs, trace=False)
    return out
